# revision 19
# baseline (speedup 1.0000x reference)
"""Trainium2 Bass kernel for the HFNN (hierarchical fuzzy NN) forward pass.

Math (per branch k of 8, rule r of 32, feature f of 16, batch b of 32768):
  expo[k,b,r]  = sum_f (x-mu)^2 / (2 sigma^2)
  E            = exp(-expo);  normalized over r
  conq[k,b,r]  = w3_bias + sum_f w3 * x
  tsk[k,b]     = sum_r E*conq / sum_r E
  out          = softmax(w5 @ tsk + b5) over 2 classes

Device strategy (pure batch data-parallel over 8 cores, 4096 batch each):
  - Host ships per core two fp32r slabs S_g [128, 4096] (g = branch group of
    4): rows 32i+{0..15} = x^2, rows 32i+{16..31} = x for branch 4g+i.
  - Per 512-col chunk and group: one fp32r matmul (block-diag weights) gives
    m = -expo + const in PSUM; ACT computes E = exp(m + bias) (fp32r, two
    chunks per activation); a second fp32r matmul gives conq0 = w3.x; DVE
    computes EC = E * conq0.
  - fp32r reduction matmuls with host-padded M=128 weights accumulate
    den = sum_r E, numb = sum_r E*w3bias, num0 = sum_r EC for 4 chunks into
    one PSUM collector bank (rows 32q+{0-7,8-15,16-23}); ACT copies it to
    SBUF; a strided DMA ships only the used rows.
  - Host does the remaining O(B) work exactly in float64: num = num0 + numb,
    tsk = num/den, d = (w5[0]-w5[1]).tsk + (b5[0]-b5[1]), p = sigmoid(+-d).
"""

import numpy as np

import concourse.bacc as bacc
import concourse.tile as tile
from concourse import mybir
from concourse.bass_utils import run_bass_kernel_spmd

F32 = mybir.dt.float32
F32R = mybir.dt.float32r

NB, NR, NF = 8, 32, 16
NBATCH, NCORE = 32768, 8
BC = NBATCH // NCORE          # 4096 batch per core
CH = 512                      # chunk (psum bank) width
NCH = BC // CH                # 8 chunks
NROUND = 4                    # collector rounds
CPR = NCH // NROUND           # chunks per round = 4
NSC = NCH // 2                # superchunks (1024 wide) = 4

_CACHE: dict = {}


def _build_nc():
    nc = bacc.Bacc("TRN2", target_bir_lowering=False, debug=False)
    s_in = [
        nc.dram_tensor(f"s{g}", [128, BC], F32R, kind="ExternalInput")
        for g in range(2)
    ]
    # main weights: wm0 | wm1 | wc0 | wc1
    wall_in = nc.dram_tensor("wall", [128, 512], F32R, kind="ExternalInput")
    # padded reduction weights: 8 x [128, 128], idx = 4*g + 2*q + kind
    wpad_in = nc.dram_tensor("wpad", [128, 1024], F32R, kind="ExternalInput")
    bias_in = nc.dram_tensor("ebias", [128, 2], F32, kind="ExternalInput")
    out_c = nc.dram_tensor("outc", [NROUND, 48, CH], F32, kind="ExternalOutput")

    with tile.TileContext(nc) as tc:
        with (
            tc.tile_pool(name="wpool", bufs=1) as wpool,
            tc.tile_pool(name="spool", bufs=8) as spool,
            tc.tile_pool(name="epool", bufs=10) as epool,
            tc.tile_pool(name="opool", bufs=2) as opool,
            tc.tile_pool(name="mps", bufs=2, space="PSUM") as mps,
            tc.tile_pool(name="cps", bufs=2, space="PSUM") as cps,
            tc.tile_pool(name="collps", bufs=2, space="PSUM") as collps,
        ):
            wall = wpool.tile([128, 512], F32R, tag="wall")
            nc.gpsimd.dma_start(out=wall[:], in_=wall_in[:, :])
            wpad = wpool.tile([128, 1024], F32R, tag="wpad")
            nc.gpsimd.dma_start(out=wpad[:], in_=wpad_in[:, :])
            bias_t = wpool.tile([128, 2], F32, tag="bias")
            nc.gpsimd.dma_start(out=bias_t[:], in_=bias_in[:, :])

            warm = wpool.tile([128, 1], F32, tag="warm")
            nc.vector.memset(warm[:], 0.0)
            nc.scalar.activation(
                warm[:], warm[:], mybir.ActivationFunctionType.Exp
            )

            def wm(g):
                return wall[:, 128 * g : 128 * (g + 1)]

            def wc(g):
                return wall[:, 256 + 128 * g : 256 + 128 * (g + 1)]

            def wred(g, q, kind):
                i = 4 * g + 2 * q + kind
                return wpad[:, 128 * i : 128 * (i + 1)]

            for sc in range(NSC):
                rnd = sc
                coll = collps.tile([128, CH], F32, tag="coll", name=f"coll{rnd}")
                m_t, s_t = [], []
                for g in range(2):
                    mt_g = mps.tile([128, 1024], F32, tag="m", name=f"m{sc}{g}")
                    m_t.append(mt_g)
                for h in range(2):
                    j = 2 * sc + h
                    for g in range(2):
                        st = spool.tile([128, CH], F32R, tag="s")
                        nc.sync.dma_start(
                            out=st[:], in_=s_in[g][:, j * CH : (j + 1) * CH]
                        )
                        s_t.append(st)
                        nc.tensor.matmul(
                            m_t[g][:, h * CH : (h + 1) * CH],
                            wm(g), st[:], start=True, stop=True,
                        )
                e_t = []
                for g in range(2):
                    et = epool.tile([128, 1024], F32R, tag="e")
                    nc.scalar.activation(
                        et[:], m_t[g][:], mybir.ActivationFunctionType.Exp,
                        bias=bias_t[:, g : g + 1], scale=1.0,
                    )
                    e_t.append(et)
                ec_t = [epool.tile([128, 1024], F32R, tag="ec", name=f"ec{sc}{gg}") for gg in range(2)]
                for h in range(2):
                    j = 2 * sc + h
                    q = h
                    first = (q == 0)
                    for g in range(2):
                        c_ps = cps.tile([128, CH], F32, tag="c")
                        nc.tensor.matmul(
                            c_ps[:], wc(g), s_t[2 * h + g][:],
                            start=True, stop=True,
                        )
                        nc.vector.tensor_mul(
                            ec_t[g][:, h * CH : (h + 1) * CH],
                            e_t[g][:, h * CH : (h + 1) * CH],
                            c_ps[:],
                        )
                    for g in range(2):
                        nc.tensor.matmul(
                            coll[:], wred(g, q, 0),
                            e_t[g][:, h * CH : (h + 1) * CH],
                            start=(first and g == 0), stop=False,
                        )
                        last = (q == 1) and (g == 1)
                        nc.tensor.matmul(
                            coll[:], wred(g, q, 1),
                            ec_t[g][:, h * CH : (h + 1) * CH],
                            start=False, stop=last,
                        )
                ot = opool.tile([128, CH], F32, tag="o")
                if sc % 2 == 0:
                    nc.scalar.copy(ot[:48], coll[:48])
                else:
                    nc.vector.tensor_copy(ot[:48], coll[:48])
                nc.scalar.dma_start(out=out_c[rnd], in_=ot[:48])
    nc.finalize()
    return nc


def _host_prep(data, para_mu, para_sigma, para_w3):
    xt = np.ascontiguousarray(data.transpose(0, 2, 1)).astype(np.float32)
    x2t = xt * xt
    slabs = []
    for g in range(2):
        s = np.empty((128, NBATCH), np.float32)
        for i in range(4):
            k = 4 * g + i
            s[32 * i : 32 * i + 16] = x2t[k]
            s[32 * i + 16 : 32 * i + 32] = xt[k]
        slabs.append(s)

    sig2 = para_sigma.astype(np.float64) ** 2
    mu = para_mu.astype(np.float64)
    a_neg = -1.0 / (2.0 * sig2)                     # [8, 32, 16]
    m2 = mu / sig2
    c = np.sum(mu * mu / (2.0 * sig2), axis=-1)     # [8, 32]

    wall = np.zeros((128, 512), np.float32)
    ebias = np.zeros((128, 2), np.float32)
    for g in range(2):
        for i in range(4):
            k = 4 * g + i
            r0, rf = 32 * i, 32 * i + 16
            wall[r0 : r0 + 16, 128 * g + r0 : 128 * g + r0 + 32] = a_neg[k].T
            wall[rf : rf + 16, 128 * g + r0 : 128 * g + r0 + 32] = m2[k].T
            wall[rf : rf + 16, 256 + 128 * g + r0 : 256 + 128 * g + r0 + 32] = (
                para_w3[k, :, :NF].T
            )
            ebias[r0 : r0 + 32, g] = -c[k]

    wpad = np.zeros((128, 1024), np.float32)
    for g in range(2):
        for q in range(2):
            for i in range(4):
                k = 4 * g + i
                rows = slice(32 * i, 32 * i + 32)
                c_red = 128 * (4 * g + 2 * q)
                c_num = 128 * (4 * g + 2 * q + 1)
                wpad[rows, c_red + 24 * q + k] = 1.0
                wpad[rows, c_red + 24 * q + 8 + k] = para_w3[k, :, NF]
                wpad[rows, c_num + 24 * q + 16 + k] = 1.0
    return slabs, wall, wpad, ebias


def kernel(data, para_mu, para_sigma, para_w3, w5, b5):
    if "nc" not in _CACHE:
        _CACHE["nc"] = _build_nc()
    nc = _CACHE["nc"]

    slabs, wall, wpad, ebias = _host_prep(data, para_mu, para_sigma, para_w3)
    in_maps = []
    for cidx in range(NCORE):
        cols = slice(cidx * BC, (cidx + 1) * BC)
        in_maps.append(
            {
                "s0": np.ascontiguousarray(slabs[0][:, cols]),
                "s1": np.ascontiguousarray(slabs[1][:, cols]),
                "wall": wall,
                "wpad": wpad,
                "ebias": ebias,
            }
        )
    try:
        res = run_bass_kernel_spmd(nc, in_maps, core_ids=list(range(NCORE)))
    except Exception:
        # transient NRT device errors (e.g. a wedged core) recover on retry
        res = run_bass_kernel_spmd(nc, in_maps, core_ids=list(range(NCORE)))
    _CACHE["last_result"] = res

    # ---- host epilogue (exact, O(B)) ----
    den = np.empty((NB, NBATCH), np.float64)
    numb = np.empty((NB, NBATCH), np.float64)
    num0 = np.empty((NB, NBATCH), np.float64)
    for cidx in range(NCORE):
        arr = res.results[cidx]["outc"].astype(np.float64)  # [4, 48, 512]
        v = np.moveaxis(arr.reshape(NROUND, 2, 24, CH), 2, 0)
        v = v.reshape(24, BC)  # row l, local batch (rnd, q, t)
        cols = slice(cidx * BC, (cidx + 1) * BC)
        den[:, cols] = v[0:8]
        numb[:, cols] = v[8:16]
        num0[:, cols] = v[16:24]

    tsk = (num0 + numb) / den                     # [8, B]
    w5d = (w5[0] - w5[1]).astype(np.float64)
    d = w5d @ tsk + (float(b5[0]) - float(b5[1]))
    p0 = 1.0 / (1.0 + np.exp(-d))
    out = np.empty((NBATCH, 2), np.float32)
    out[:, 0] = p0.astype(np.float32)
    out[:, 1] = (1.0 - p0).astype(np.float32)
    return out


# revision 21
# speedup vs baseline: 1.0198x; 1.0198x over previous
"""Trainium2 Bass kernel for the HFNN (hierarchical fuzzy NN) forward pass.

Math (per branch k of 8, rule r of 32, feature f of 16, batch b of 32768):
  expo[k,b,r]  = sum_f (x-mu)^2 / (2 sigma^2)
  E            = exp(-expo);  normalized over r
  conq[k,b,r]  = w3_bias + sum_f w3 * x
  tsk[k,b]     = sum_r E*conq / sum_r E
  out          = softmax(w5 @ tsk + b5) over 2 classes

Device strategy (pure batch data-parallel over 8 cores, 4096 batch each):
  - Host ships per core two fp32r slabs S_g [128, 4096] (g = branch group of
    4): rows 32i+{0..15} = x^2, rows 32i+{16..31} = x for branch 4g+i.
  - Per 512-col chunk and group: one fp32r matmul (block-diag weights) gives
    m = -expo + const in PSUM; ACT computes E = exp(m + bias) (fp32r, two
    chunks per activation); a second fp32r matmul gives conq0 = w3.x; DVE
    computes EC = E * conq0.
  - fp32r reduction matmuls with host-padded M=128 weights accumulate
    den = sum_r E, numb = sum_r E*w3bias, num0 = sum_r EC for 4 chunks into
    one PSUM collector bank (rows 32q+{0-7,8-15,16-23}); ACT copies it to
    SBUF; a strided DMA ships only the used rows.
  - Host does the remaining O(B) work exactly in float64: num = num0 + numb,
    tsk = num/den, d = (w5[0]-w5[1]).tsk + (b5[0]-b5[1]), p = sigmoid(+-d).
"""

import numpy as np

import concourse.bacc as bacc
import concourse.tile as tile
from concourse import mybir
from concourse.bass_utils import run_bass_kernel_spmd

F32 = mybir.dt.float32
F32R = mybir.dt.float32r

NB, NR, NF = 8, 32, 16
NBATCH, NCORE = 32768, 8
BC = NBATCH // NCORE          # 4096 batch per core
CH = 512                      # chunk (psum bank) width
NCH = BC // CH                # 8 chunks
NROUND = 4                    # collector rounds
CPR = NCH // NROUND           # chunks per round = 4
NSC = NCH // 2                # superchunks (1024 wide) = 4

_CACHE: dict = {}


def _build_nc():
    nc = bacc.Bacc("TRN2", target_bir_lowering=False, debug=False)
    s_in = [
        nc.dram_tensor(f"s{g}", [128, BC], F32R, kind="ExternalInput")
        for g in range(2)
    ]
    # main weights: wm0 | wm1 | wc0 | wc1
    wall_in = nc.dram_tensor("wall", [128, 512], F32R, kind="ExternalInput")
    # padded reduction weights: 8 x [128, 128], idx = 4*g + 2*q + kind
    wpad_in = nc.dram_tensor("wpad", [128, 1024], F32R, kind="ExternalInput")
    bias_in = nc.dram_tensor("ebias", [128, 2], F32, kind="ExternalInput")
    out_c = nc.dram_tensor("outc", [NROUND, 48, CH], F32, kind="ExternalOutput")

    with tile.TileContext(nc) as tc:
        with (
            tc.tile_pool(name="wpool", bufs=1) as wpool,
            tc.tile_pool(name="spool", bufs=12) as spool,
            tc.tile_pool(name="epool", bufs=12) as epool,
            tc.tile_pool(name="opool", bufs=4) as opool,
            tc.tile_pool(name="mps", bufs=2, space="PSUM") as mps,
            tc.tile_pool(name="cps", bufs=2, space="PSUM") as cps,
            tc.tile_pool(name="collps", bufs=2, space="PSUM") as collps,
        ):
            wall = wpool.tile([128, 512], F32R, tag="wall")
            nc.gpsimd.dma_start(out=wall[:], in_=wall_in[:, :])
            wpad = wpool.tile([128, 1024], F32R, tag="wpad")
            nc.gpsimd.dma_start(out=wpad[:], in_=wpad_in[:, :])
            bias_t = wpool.tile([128, 2], F32, tag="bias")
            nc.gpsimd.dma_start(out=bias_t[:], in_=bias_in[:, :])

            warm = wpool.tile([128, 1], F32, tag="warm")
            nc.vector.memset(warm[:], 0.0)
            nc.scalar.activation(
                warm[:], warm[:], mybir.ActivationFunctionType.Exp
            )

            def wm(g):
                return wall[:, 128 * g : 128 * (g + 1)]

            def wc(g):
                return wall[:, 256 + 128 * g : 256 + 128 * (g + 1)]

            def wred(g, q, kind):
                i = 4 * g + 2 * q + kind
                return wpad[:, 128 * i : 128 * (i + 1)]

            for sc in range(NSC):
                rnd = sc
                coll = collps.tile([128, CH], F32, tag="coll", name=f"coll{rnd}")
                m_t, s_t = [], []
                for g in range(2):
                    mt_g = mps.tile([128, 1024], F32, tag="m", name=f"m{sc}{g}")
                    m_t.append(mt_g)
                for h in range(2):
                    j = 2 * sc + h
                    for g in range(2):
                        st = spool.tile([128, CH], F32R, tag="s")
                        nc.sync.dma_start(
                            out=st[:], in_=s_in[g][:, j * CH : (j + 1) * CH]
                        )
                        s_t.append(st)
                        nc.tensor.matmul(
                            m_t[g][:, h * CH : (h + 1) * CH],
                            wm(g), st[:], start=True, stop=True,
                        )
                e_t = []
                for g in range(2):
                    et = epool.tile([128, 1024], F32R, tag="e")
                    nc.scalar.activation(
                        et[:], m_t[g][:], mybir.ActivationFunctionType.Exp,
                        bias=bias_t[:, g : g + 1], scale=1.0,
                    )
                    e_t.append(et)
                ec_t = [epool.tile([128, 1024], F32R, tag="ec", name=f"ec{sc}{gg}") for gg in range(2)]
                for h in range(2):
                    j = 2 * sc + h
                    q = h
                    first = (q == 0)
                    for g in range(2):
                        c_ps = cps.tile([128, CH], F32, tag="c")
                        nc.tensor.matmul(
                            c_ps[:], wc(g), s_t[2 * h + g][:],
                            start=True, stop=True,
                        )
                        nc.vector.tensor_mul(
                            ec_t[g][:, h * CH : (h + 1) * CH],
                            e_t[g][:, h * CH : (h + 1) * CH],
                            c_ps[:],
                        )
                    for g in range(2):
                        nc.tensor.matmul(
                            coll[:], wred(g, q, 0),
                            e_t[g][:, h * CH : (h + 1) * CH],
                            start=(first and g == 0), stop=False,
                        )
                        last = (q == 1) and (g == 1)
                        nc.tensor.matmul(
                            coll[:], wred(g, q, 1),
                            ec_t[g][:, h * CH : (h + 1) * CH],
                            start=False, stop=last,
                        )
                ot = opool.tile([128, CH], F32, tag="o")
                if sc % 2 == 0:
                    nc.scalar.copy(ot[:48], coll[:48])
                else:
                    nc.vector.tensor_copy(ot[:48], coll[:48])
                nc.scalar.dma_start(out=out_c[rnd], in_=ot[:48])
    nc.finalize()
    return nc


def _host_prep(data, para_mu, para_sigma, para_w3):
    xt = np.ascontiguousarray(data.transpose(0, 2, 1)).astype(np.float32)
    x2t = xt * xt
    slabs = []
    for g in range(2):
        s = np.empty((128, NBATCH), np.float32)
        for i in range(4):
            k = 4 * g + i
            s[32 * i : 32 * i + 16] = x2t[k]
            s[32 * i + 16 : 32 * i + 32] = xt[k]
        slabs.append(s)

    sig2 = para_sigma.astype(np.float64) ** 2
    mu = para_mu.astype(np.float64)
    a_neg = -1.0 / (2.0 * sig2)                     # [8, 32, 16]
    m2 = mu / sig2
    c = np.sum(mu * mu / (2.0 * sig2), axis=-1)     # [8, 32]

    wall = np.zeros((128, 512), np.float32)
    ebias = np.zeros((128, 2), np.float32)
    for g in range(2):
        for i in range(4):
            k = 4 * g + i
            r0, rf = 32 * i, 32 * i + 16
            wall[r0 : r0 + 16, 128 * g + r0 : 128 * g + r0 + 32] = a_neg[k].T
            wall[rf : rf + 16, 128 * g + r0 : 128 * g + r0 + 32] = m2[k].T
            wall[rf : rf + 16, 256 + 128 * g + r0 : 256 + 128 * g + r0 + 32] = (
                para_w3[k, :, :NF].T
            )
            ebias[r0 : r0 + 32, g] = -c[k]

    wpad = np.zeros((128, 1024), np.float32)
    for g in range(2):
        for q in range(2):
            for i in range(4):
                k = 4 * g + i
                rows = slice(32 * i, 32 * i + 32)
                c_red = 128 * (4 * g + 2 * q)
                c_num = 128 * (4 * g + 2 * q + 1)
                wpad[rows, c_red + 24 * q + k] = 1.0
                wpad[rows, c_red + 24 * q + 8 + k] = para_w3[k, :, NF]
                wpad[rows, c_num + 24 * q + 16 + k] = 1.0
    return slabs, wall, wpad, ebias


def kernel(data, para_mu, para_sigma, para_w3, w5, b5):
    if "nc" not in _CACHE:
        _CACHE["nc"] = _build_nc()
    nc = _CACHE["nc"]

    slabs, wall, wpad, ebias = _host_prep(data, para_mu, para_sigma, para_w3)
    in_maps = []
    for cidx in range(NCORE):
        cols = slice(cidx * BC, (cidx + 1) * BC)
        in_maps.append(
            {
                "s0": np.ascontiguousarray(slabs[0][:, cols]),
                "s1": np.ascontiguousarray(slabs[1][:, cols]),
                "wall": wall,
                "wpad": wpad,
                "ebias": ebias,
            }
        )
    try:
        res = run_bass_kernel_spmd(nc, in_maps, core_ids=list(range(NCORE)))
    except Exception:
        # transient NRT device errors (e.g. a wedged core) recover on retry
        res = run_bass_kernel_spmd(nc, in_maps, core_ids=list(range(NCORE)))
    _CACHE["last_result"] = res

    # ---- host epilogue (exact, O(B)) ----
    den = np.empty((NB, NBATCH), np.float64)
    numb = np.empty((NB, NBATCH), np.float64)
    num0 = np.empty((NB, NBATCH), np.float64)
    for cidx in range(NCORE):
        arr = res.results[cidx]["outc"].astype(np.float64)  # [4, 48, 512]
        v = np.moveaxis(arr.reshape(NROUND, 2, 24, CH), 2, 0)
        v = v.reshape(24, BC)  # row l, local batch (rnd, q, t)
        cols = slice(cidx * BC, (cidx + 1) * BC)
        den[:, cols] = v[0:8]
        numb[:, cols] = v[8:16]
        num0[:, cols] = v[16:24]

    tsk = (num0 + numb) / den                     # [8, B]
    w5d = (w5[0] - w5[1]).astype(np.float64)
    d = w5d @ tsk + (float(b5[0]) - float(b5[1]))
    p0 = 1.0 / (1.0 + np.exp(-d))
    out = np.empty((NBATCH, 2), np.float32)
    out[:, 0] = p0.astype(np.float32)
    out[:, 1] = (1.0 - p0).astype(np.float32)
    return out


# revision 27
# speedup vs baseline: 1.0327x; 1.0127x over previous
"""Trainium2 Bass kernel for the HFNN (hierarchical fuzzy NN) forward pass.

Math (per branch k of 8, rule r of 32, feature f of 16, batch b of 32768):
  expo[k,b,r]  = sum_f (x-mu)^2 / (2 sigma^2)
  E            = exp(-expo);  normalized over r
  conq[k,b,r]  = w3_bias + sum_f w3 * x
  tsk[k,b]     = sum_r E*conq / sum_r E
  out          = softmax(w5 @ tsk + b5) over 2 classes

Device strategy (pure batch data-parallel over 8 cores, 4096 batch each):
  - Host ships per core two fp32r slabs S_g [128, 4096] (g = branch group of
    4): rows 32i+{0..15} = x^2, rows 32i+{16..31} = x for branch 4g+i.
  - Per 512-col chunk and group: one fp32r matmul (block-diag weights) gives
    m = -expo + const in PSUM; ACT computes E = exp(m + bias) (fp32r, two
    chunks per activation); a second fp32r matmul gives conq0 = w3.x; DVE
    computes EC = E * conq0.
  - fp32r reduction matmuls with host-padded M=128 weights accumulate
    den = sum_r E, numb = sum_r E*w3bias, num0 = sum_r EC for 4 chunks into
    one PSUM collector bank (rows 32q+{0-7,8-15,16-23}); ACT copies it to
    SBUF; a strided DMA ships only the used rows.
  - Host does the remaining O(B) work exactly in float64: num = num0 + numb,
    tsk = num/den, d = (w5[0]-w5[1]).tsk + (b5[0]-b5[1]), p = sigmoid(+-d).
"""

import numpy as np

import concourse.bacc as bacc
import concourse.tile as tile
from concourse import mybir
from concourse.bass_utils import run_bass_kernel_spmd

F32 = mybir.dt.float32
F32R = mybir.dt.float32r

NB, NR, NF = 8, 32, 16
NBATCH, NCORE = 32768, 8
BC = NBATCH // NCORE          # 4096 batch per core
CH = 512                      # chunk (psum bank) width
NCH = BC // CH                # 8 chunks
NROUND = 4                    # collector rounds
CPR = NCH // NROUND           # chunks per round = 4
NSC = NCH // 2                # superchunks (1024 wide) = 4

_CACHE: dict = {}


def _build_nc():
    nc = bacc.Bacc("TRN2", target_bir_lowering=False, debug=False)
    s_in = [
        nc.dram_tensor(f"s{g}", [128, BC], F32R, kind="ExternalInput")
        for g in range(2)
    ]
    # main weights: wm0 | wm1 | wc0 | wc1
    wall_in = nc.dram_tensor("wall", [128, 512], F32R, kind="ExternalInput")
    # padded reduction weights: 8 x [128, 128], idx = 4*g + 2*q + kind
    wpad_in = nc.dram_tensor("wpad", [128, 1024], F32R, kind="ExternalInput")
    bias_in = nc.dram_tensor("ebias", [128, 2], F32, kind="ExternalInput")
    out_c = nc.dram_tensor("outc", [NROUND, 48, CH], F32, kind="ExternalOutput")

    with tile.TileContext(nc) as tc:
        with (
            tc.tile_pool(name="wpool", bufs=1) as wpool,
            tc.tile_pool(name="spool", bufs=12) as spool,
            tc.tile_pool(name="epool", bufs=12) as epool,
            tc.tile_pool(name="opool", bufs=4) as opool,
            tc.tile_pool(name="mps", bufs=4, space="PSUM") as mps,
            tc.tile_pool(name="cps", bufs=2, space="PSUM") as cps,
            tc.tile_pool(name="collps", bufs=2, space="PSUM") as collps,
        ):
            wall = wpool.tile([128, 512], F32R, tag="wall")
            nc.gpsimd.dma_start(out=wall[:], in_=wall_in[:, :])
            wpad = wpool.tile([128, 1024], F32R, tag="wpad")
            nc.gpsimd.dma_start(out=wpad[:], in_=wpad_in[:, :])
            bias_t = wpool.tile([128, 2], F32, tag="bias")
            nc.gpsimd.dma_start(out=bias_t[:], in_=bias_in[:, :])

            warm = wpool.tile([128, 1], F32, tag="warm")
            nc.vector.memset(warm[:], 0.0)
            nc.scalar.activation(
                warm[:], warm[:], mybir.ActivationFunctionType.Exp
            )

            def wm(g):
                return wall[:, 128 * g : 128 * (g + 1)]

            def wc(g):
                return wall[:, 256 + 128 * g : 256 + 128 * (g + 1)]

            def wred(g, q, kind):
                i = 4 * g + 2 * q + kind
                return wpad[:, 128 * i : 128 * (i + 1)]

            for sc in range(NSC):
                rnd = sc
                coll = collps.tile([128, CH], F32, tag="coll", name=f"coll{rnd}")
                s_t = []
                e_t = [
                    epool.tile([128, 1024], F32R, tag="e", name=f"e{sc}{gg}")
                    for gg in range(2)
                ]
                for h in range(2):
                    j = 2 * sc + h
                    for g in range(2):
                        st = spool.tile([128, CH], F32R, tag="s")
                        nc.sync.dma_start(
                            out=st[:], in_=s_in[g][:, j * CH : (j + 1) * CH]
                        )
                        s_t.append(st)
                        mt = mps.tile([128, CH], F32, tag="m", name=f"m{j}{g}")
                        nc.tensor.matmul(
                            mt[:], wm(g), st[:], start=True, stop=True,
                        )
                        nc.scalar.activation(
                            e_t[g][:, h * CH : (h + 1) * CH], mt[:],
                            mybir.ActivationFunctionType.Exp,
                            bias=bias_t[:, g : g + 1], scale=1.0,
                        )
                ec_t = [epool.tile([128, 1024], F32R, tag="ec", name=f"ec{sc}{gg}") for gg in range(2)]
                for h in range(2):
                    j = 2 * sc + h
                    q = h
                    first = (q == 0)
                    for g in range(2):
                        c_ps = cps.tile([128, CH], F32, tag="c")
                        nc.tensor.matmul(
                            c_ps[:], wc(g), s_t[2 * h + g][:],
                            start=True, stop=True,
                        )
                        nc.vector.tensor_mul(
                            ec_t[g][:, h * CH : (h + 1) * CH],
                            e_t[g][:, h * CH : (h + 1) * CH],
                            c_ps[:],
                        )
                    for g in range(2):
                        nc.tensor.matmul(
                            coll[:], wred(g, q, 0),
                            e_t[g][:, h * CH : (h + 1) * CH],
                            start=(first and g == 0), stop=False,
                        )
                        last = (q == 1) and (g == 1)
                        nc.tensor.matmul(
                            coll[:], wred(g, q, 1),
                            ec_t[g][:, h * CH : (h + 1) * CH],
                            start=False, stop=last,
                        )
                ot = opool.tile([128, CH], F32, tag="o")
                if sc % 2 == 0:
                    nc.scalar.copy(ot[:48], coll[:48])
                else:
                    nc.vector.tensor_copy(ot[:48], coll[:48])
                nc.scalar.dma_start(out=out_c[rnd], in_=ot[:48])
    nc.finalize()
    return nc


def _host_prep(data, para_mu, para_sigma, para_w3):
    xt = np.ascontiguousarray(data.transpose(0, 2, 1)).astype(np.float32)
    x2t = xt * xt
    slabs = []
    for g in range(2):
        s = np.empty((128, NBATCH), np.float32)
        for i in range(4):
            k = 4 * g + i
            s[32 * i : 32 * i + 16] = x2t[k]
            s[32 * i + 16 : 32 * i + 32] = xt[k]
        slabs.append(s)

    sig2 = para_sigma.astype(np.float64) ** 2
    mu = para_mu.astype(np.float64)
    a_neg = -1.0 / (2.0 * sig2)                     # [8, 32, 16]
    m2 = mu / sig2
    c = np.sum(mu * mu / (2.0 * sig2), axis=-1)     # [8, 32]

    wall = np.zeros((128, 512), np.float32)
    ebias = np.zeros((128, 2), np.float32)
    for g in range(2):
        for i in range(4):
            k = 4 * g + i
            r0, rf = 32 * i, 32 * i + 16
            wall[r0 : r0 + 16, 128 * g + r0 : 128 * g + r0 + 32] = a_neg[k].T
            wall[rf : rf + 16, 128 * g + r0 : 128 * g + r0 + 32] = m2[k].T
            wall[rf : rf + 16, 256 + 128 * g + r0 : 256 + 128 * g + r0 + 32] = (
                para_w3[k, :, :NF].T
            )
            ebias[r0 : r0 + 32, g] = -c[k]

    wpad = np.zeros((128, 1024), np.float32)
    for g in range(2):
        for q in range(2):
            for i in range(4):
                k = 4 * g + i
                rows = slice(32 * i, 32 * i + 32)
                c_red = 128 * (4 * g + 2 * q)
                c_num = 128 * (4 * g + 2 * q + 1)
                wpad[rows, c_red + 24 * q + k] = 1.0
                wpad[rows, c_red + 24 * q + 8 + k] = para_w3[k, :, NF]
                wpad[rows, c_num + 24 * q + 16 + k] = 1.0
    return slabs, wall, wpad, ebias


def kernel(data, para_mu, para_sigma, para_w3, w5, b5):
    if "nc" not in _CACHE:
        _CACHE["nc"] = _build_nc()
    nc = _CACHE["nc"]

    slabs, wall, wpad, ebias = _host_prep(data, para_mu, para_sigma, para_w3)
    in_maps = []
    for cidx in range(NCORE):
        cols = slice(cidx * BC, (cidx + 1) * BC)
        in_maps.append(
            {
                "s0": np.ascontiguousarray(slabs[0][:, cols]),
                "s1": np.ascontiguousarray(slabs[1][:, cols]),
                "wall": wall,
                "wpad": wpad,
                "ebias": ebias,
            }
        )
    try:
        res = run_bass_kernel_spmd(nc, in_maps, core_ids=list(range(NCORE)))
    except Exception:
        # transient NRT device errors (e.g. a wedged core) recover on retry
        res = run_bass_kernel_spmd(nc, in_maps, core_ids=list(range(NCORE)))
    _CACHE["last_result"] = res

    # ---- host epilogue (exact, O(B)) ----
    den = np.empty((NB, NBATCH), np.float64)
    numb = np.empty((NB, NBATCH), np.float64)
    num0 = np.empty((NB, NBATCH), np.float64)
    for cidx in range(NCORE):
        arr = res.results[cidx]["outc"].astype(np.float64)  # [4, 48, 512]
        v = np.moveaxis(arr.reshape(NROUND, 2, 24, CH), 2, 0)
        v = v.reshape(24, BC)  # row l, local batch (rnd, q, t)
        cols = slice(cidx * BC, (cidx + 1) * BC)
        den[:, cols] = v[0:8]
        numb[:, cols] = v[8:16]
        num0[:, cols] = v[16:24]

    tsk = (num0 + numb) / den                     # [8, B]
    w5d = (w5[0] - w5[1]).astype(np.float64)
    d = w5d @ tsk + (float(b5[0]) - float(b5[1]))
    p0 = 1.0 / (1.0 + np.exp(-d))
    out = np.empty((NBATCH, 2), np.float32)
    out[:, 0] = p0.astype(np.float32)
    out[:, 1] = (1.0 - p0).astype(np.float32)
    return out
